# revision 17
# baseline (speedup 1.0000x reference)
"""BarycenterNorm (eval mode) Trainium2 kernel.

Math: the reference chain collapses to out_i = exp(T log(T X_i T^T) T^T)
with T = chol(B^-1).T (verified vs reference to 5e-9). With X and L
symmetric, both congruences avoid per-sample transposes on the PE
(out = lhsT.T @ rhs):
  A1 = X_i T^T    per-sample stationary X_i, shared moving T^T  (f32 quad)
  Y  = T A1       shared stationary diag(aT^T,aT^T), f32r       -> W
  W  = a*Y + b*I  spectrum in [-1,1]
  2L = cheb2-Clenshaw(W)    all-bf16, two-level Chebyshev basis:
       p(W) = sum_j Q_j(W) T_j(z), z = T_s(W); doubled recurrence
       Bt_j = 2(z Bt_{j+1} + Q_j) - Bt_{j+2} keeps the x2 out of Q.
  A2 = (2L) T^T   per-sample stationary, f32 quad
  V  = T A2/(2zb) shared f32r; spectrum [-1,1]
  out = p_exp(V)  monomial PS (coeffs O(1)), bf16

Layout: 16-sample groups; tiles [128,512]: samples 16g..16g+7 in
partitions 0-63, 16g+8..16g+15 in 64-127, each a [64,64] block along
free. Per-sample matmuls are 64x64 quadrant matmuls at (0,0)/(64,64).
Coefficient injection: K=128 (c*I) matmuls accumulate in PSUM; they must
be emitted BEFORE the quadrant matmuls of the same bank (HW quirk:
full-K accumulation after quadrant-mode start=True writes corrupts).
"""
import os
import sys

import numpy as np

sys.path.insert(0, "/opt/trn_rl_repo")

import concourse.bacc as bacc  # noqa: E402
import concourse.tile as tile  # noqa: E402
from concourse import mybir  # noqa: E402
from concourse import bass_utils  # noqa: E402

try:
    import axon_profile_shim  # noqa: F401
except Exception:
    pass

F32 = mybir.dt.float32
F32R = mybir.dt.float32r
BF16 = mybir.dt.bfloat16

C = 64
BATCH = 8192
NCORES = 8
SPC = BATCH // NCORES
NGRP = SPC // 16

N_LOG = int(os.environ.get("K_NLOG", 13))
S_LOG = int(os.environ.get("K_SLOG", 4))
N_EXP = int(os.environ.get("K_NEXP", 4))
S_EXP = int(os.environ.get("K_SEXP", 3))
A_MARGIN = 0.98
B_MARGIN = 6.5

MULT = mybir.AluOpType.mult
ADD = mybir.AluOpType.add
SUB = mybir.AluOpType.subtract
COPYF = mybir.ActivationFunctionType.Copy


def _cheb_coeffs(f, lo, hi, deg):
    k = np.arange(deg + 1)
    nw = np.cos((2 * k + 1) * np.pi / (2 * (deg + 1)))
    nx = 0.5 * (hi - lo) * nw + 0.5 * (lo + hi)
    return np.polynomial.chebyshev.chebfit(nw, f(nx), deg)


def _solve_two_level(a, s):
    """p = sum_{j,i} b[j,i] T_i(w) T_{js}(w); triangular solve."""
    n = len(a) - 1
    r = n // s
    rem = a.astype(np.float64).copy()
    b = np.zeros((r + 1, s))
    for j in range(r, -1, -1):
        for i in range(min(s - 1, n - j * s), 0, -1):
            m = j * s + i
            if j == 0:
                b[j, i] = rem[m]
                rem[m] = 0.0
            else:
                coef = 2.0 * rem[m]
                b[j, i] = coef
                rem[m] = 0.0
                rem[abs(j * s - i)] -= coef / 2.0
        b[j, 0] = rem[j * s]
        rem[j * s] = 0.0
    assert np.abs(rem).max() < 1e-10
    return b


def _host_prep(running_mean):
    B = running_mean[0].astype(np.float64)
    sev = np.linalg.eigvalsh(B)
    T = np.linalg.cholesky(np.linalg.inv(B)).T  # upper; T^T T = B^-1
    a = A_MARGIN / sev[-1]
    b = B_MARGIN / sev[0]
    alpha = 2.0 / (b - a)
    beta = -(a + b) / (b - a)
    zb = max(abs(np.log(a)), abs(np.log(b))) / sev[0]
    b_log = _solve_two_level(_cheb_coeffs(np.log, a, b, N_LOG), S_LOG)
    c_exp = np.polynomial.chebyshev.cheb2poly(
        _cheb_coeffs(lambda v: np.exp(zb * v / 2.0), -1.0, 1.0, N_EXP))
    return dict(T=T, alpha=alpha, beta=beta, zb=zb, b_log=b_log,
                c_exp=c_exp, r_log=N_LOG // S_LOG, r_exp=N_EXP // S_EXP)


def _build_consts(hp):
    T = hp["T"].astype(np.float32)
    Tt = np.ascontiguousarray(T.T)
    alpha = np.float32(hp["alpha"])
    zb = np.float32(hp["zb"])
    I64 = np.eye(64, dtype=np.float32)

    slabs = {}
    cols = []

    def add(name, arr):
        c0 = sum(a.shape[1] for a in cols)
        cols.append(np.ascontiguousarray(arr, dtype=np.float32))
        slabs[name] = (c0, c0 + arr.shape[1])

    dTT = np.zeros((128, 128), np.float32)
    dTT[:64, :64] = Tt
    dTT[64:, 64:] = Tt
    add("S1", alpha * dTT)
    add("S2", dTT / (2.0 * zb))
    add("Tmov", np.concatenate([Tt, Tt], axis=0))  # [128, 64]

    ipat = np.zeros((128, 512), np.float32)
    for j in range(8):
        ipat[:64, 64 * j:64 * j + 64] = I64
        ipat[64:, 64 * j:64 * j + 64] = I64

    add("IP_beta", np.float32(hp["beta"]) * ipat)
    add("IP_unit", ipat)

    ce = hp["c_exp"]
    add("IP_e2", np.float32(ce[2]) * ipat)
    add("IP_e0", np.float32(ce[0]) * ipat)

    I128 = np.eye(128, dtype=np.float32)
    b_log = hp["b_log"]
    add("IP_br0", np.float32(2.0 * b_log[hp["r_log"], 0]) * ipat)
    for j in range(hp["r_log"] + 1):
        for i in range(S_LOG):
            v = b_log[j, i] * (2.0 if j == 0 else 1.0)
            if j == hp["r_log"] and i == 0:
                continue  # rides on the stage-r copy-back
            if abs(v) > 3e-4:
                add(f"CL{j}_{i}", np.float32(v) * I128)
    for i in (4, 3, 1):
        add(f"CE_{i}", np.float32(ce[i]) * I128)

    return np.concatenate(cols, axis=1), slabs, None


def _build_kernel(hp, ngrp=None):
    if ngrp is None:
        ngrp = int(os.environ.get("K_NGRP", NGRP))
    blob, slabs, _ = _build_consts(hp)
    r_log, r_exp = hp["r_log"], hp["r_exp"]

    nc = bacc.Bacc("TRN2", target_bir_lowering=False, debug=False)
    x_d = nc.dram_tensor("x", [SPC, C, C], F32, kind="ExternalInput").ap()
    cst_d = nc.dram_tensor("cst", [128, blob.shape[1]], F32,
                           kind="ExternalInput").ap()
    out_d = nc.dram_tensor("out", [SPC, C, C], F32, kind="ExternalOutput").ap()

    x_r = x_d.rearrange("(g two p) r c -> g two r p c", g=NGRP, two=2)
    o_r = out_d.rearrange("(g two p) r c -> g two r p c", g=NGRP, two=2)

    with tile.TileContext(nc) as tc:
        with tc.tile_pool(name="csts", bufs=1) as csts, \
             tc.tile_pool(name="work", bufs=int(os.environ.get("K_WBUFS", 7)) ) as work, \
             tc.tile_pool(name="psp", bufs=int(os.environ.get("K_PSBUFS", 8)), space="PSUM") as psp:

            _cst_cache = {}

            def cslab(name, dtype=F32):
                c0, c1 = slabs[name]
                if name in _cst_cache:
                    t = _cst_cache[name]
                else:
                    t = csts.tile([128, c1 - c0], F32, name=f"cst_{name}",
                                  tag=f"cst_{name}")
                    nc.sync.dma_start(t, cst_d[:, c0:c1])
                    _cst_cache[name] = t
                if dtype != F32:
                    tb = csts.tile([128, c1 - c0], dtype,
                                   name=f"cstb_{name}_{dtype}",
                                   tag=f"cstb_{name}_{dtype}")
                    nc.vector.tensor_copy(tb, t)
                    return tb
                return t

            S1 = cslab("S1", F32R)
            S2 = cslab("S2", F32R)
            Tmov = cslab("Tmov")
            IPb = cslab("IP_beta")
            IPu = cslab("IP_unit")
            IPub = cslab("IP_unit", BF16)
            IP_e2 = cslab("IP_e2")
            IP_e0 = cslab("IP_e0")
            IP_br0 = cslab("IP_br0")
            CL = {}
            for j in range(r_log + 1):
                for i in range(S_LOG):
                    if f"CL{j}_{i}" in slabs:
                        CL[(j, i)] = cslab(f"CL{j}_{i}", BF16)
            CE = {i: cslab(f"CE_{i}", BF16) for i in (4, 3, 1)}


            def cb_stc(out, ps, scalar, in1, op1):
                nc.vector.scalar_tensor_tensor(out, ps, scalar, in1,
                                               MULT, op1)

            def cb_act(out, ps, scale=1.0):
                nc.scalar.activation(out, ps, COPYF, scale=scale)

            def quad16(ps, stat_tile, mov_tile, first=True, last=True,
                       mov64=None):
                # interleave row-halves so each LDWEIGHTS overlaps the
                # other half's in-flight matmul (different row_grp)
                n = 0
                for j in range(8):
                    for h in (0, 64):
                        sl = slice(64 * j, 64 * j + 64)
                        mov = (mov64[h:h + 64, 0:64] if mov64 is not None
                               else mov_tile[h:h + 64, sl])
                        nc.tensor.matmul(
                            ps[h:h + 64, sl], stat_tile[h:h + 64, sl], mov,
                            start=first, stop=(last and n == 15),
                            tile_position=(h, h))
                        n += 1

            def group_stages(g):
                st = []
                ctx = {}

                def s_load():
                    Xt = work.tile([128, 512], F32, name=f"X{g}", tag="X")
                    Xt3 = Xt.rearrange("r (p c) -> r p c", p=8)
                    nc.sync.dma_start(Xt3[0:64], x_r[g, 0])
                    nc.sync.dma_start(Xt3[64:128], x_r[g, 1])
                    ps_a = psp.tile([128, 512], F32, name=f"psa{g}", tag="ps")
                    quad16(ps_a, Xt, None, mov64=Tmov)
                    A1 = work.tile([128, 512], F32R, name=f"A1_{g}", tag="A1")
                    cb_act(A1, ps_a)
                    ctx["A1"] = A1
                st.append(s_load)

                def s_w():
                    ps_y = psp.tile([128, 512], F32, name=f"psy{g}", tag="ps")
                    nc.tensor.matmul(ps_y, S1, ctx["A1"], start=True,
                                     stop=True)
                    Wt = work.tile([128, 512], BF16, name=f"W{g}", tag="W")
                    cb_stc(Wt, ps_y, 1.0, IPb, ADD)
                    ctx["T1"] = Wt
                st.append(s_w)

                for k in range(2, S_LOG + 1):
                    def s_t(k=k):
                        Wt = ctx["T1"]
                        ps_t = psp.tile([128, 512], F32, name=f"pst{g}_{k}",
                                        tag="ps")
                        quad16(ps_t, Wt, ctx[f"T{k - 1}"] if k > 2 else Wt)
                        Tk = work.tile([128, 512], BF16, name=f"T{g}_{k}",
                                       tag=f"T{k}")
                        sub_src = IPu if k == 2 else ctx[f"T{k - 2}"]
                        cb_stc(Tk, ps_t, 2.0, sub_src, SUB)
                        ctx[f"T{k}"] = Tk
                    st.append(s_t)

                def emit_Q(ps_h, j, quad_mov=None):
                    first = True
                    for i in range(1, S_LOG):
                        if (j, i) in CL:
                            nc.tensor.matmul(ps_h, CL[(j, i)], ctx[f"T{i}"],
                                             start=first, stop=False)
                            first = False
                    if (j, 0) in CL:
                        nc.tensor.matmul(ps_h, CL[(j, 0)], IPub, start=first,
                                         stop=(quad_mov is None))
                        first = False
                    assert (not first) or quad_mov is not None
                    if quad_mov is not None:
                        quad16(ps_h, ctx[f"T{S_LOG}"], quad_mov, first=first,
                               last=True)

                def s_br():
                    ps_h = psp.tile([128, 512], F32, name=f"psq{g}_{r_log}",
                                    tag="ps")
                    emit_Q(ps_h, r_log)
                    Br = work.tile([128, 512], BF16, name=f"B{g}_{r_log}",
                                   tag=f"B{r_log}")
                    cb_stc(Br, ps_h, 2.0, IP_br0, ADD)
                    ctx[f"B{r_log}"] = Br
                st.append(s_br)

                for j in range(r_log - 1, 0, -1):
                    def s_bj(j=j):
                        ps_h = psp.tile([128, 512], F32, name=f"psq{g}_{j}",
                                        tag="ps")
                        emit_Q(ps_h, j, quad_mov=ctx[f"B{j + 1}"])
                        Bj = work.tile([128, 512], BF16, name=f"B{g}_{j}",
                                       tag=f"B{j}")
                        if f"B{j + 2}" in ctx:
                            cb_stc(Bj, ps_h, 2.0, ctx[f"B{j + 2}"], SUB)
                        else:
                            cb_act(Bj, ps_h, scale=2.0)
                        ctx[f"B{j}"] = Bj
                    st.append(s_bj)

                def s_l():
                    ps_h = psp.tile([128, 512], F32, name=f"psq{g}_0",
                                    tag="ps")
                    emit_Q(ps_h, 0, quad_mov=ctx["B1"])
                    Lt = work.tile([128, 512], F32, name=f"L{g}", tag="L")
                    cb_stc(Lt, ps_h, 1.0, ctx["B2"], SUB)
                    ctx["L"] = Lt
                st.append(s_l)

                def s_a2():
                    ps_b = psp.tile([128, 512], F32, name=f"psb{g}", tag="ps")
                    quad16(ps_b, ctx["L"], None, mov64=Tmov)
                    A2 = work.tile([128, 512], F32R, name=f"A2_{g}", tag="A2")
                    cb_act(A2, ps_b)
                    ctx["A2"] = A2
                st.append(s_a2)

                def s_v():
                    ps_v = psp.tile([128, 512], F32, name=f"psv{g}", tag="ps")
                    nc.tensor.matmul(ps_v, S2, ctx["A2"], start=True,
                                     stop=True)
                    Vt = work.tile([128, 512], BF16, name=f"V{g}", tag="V")
                    cb_act(Vt, ps_v)
                    ctx["E1"] = Vt
                st.append(s_v)

                def s_v2():
                    ps_2 = psp.tile([128, 512], F32, name=f"pv2{g}", tag="ps")
                    quad16(ps_2, ctx["E1"], ctx["E1"])
                    V2 = work.tile([128, 512], BF16, name=f"V2{g}", tag="V2")
                    cb_act(V2, ps_2)
                    ctx["E2"] = V2
                st.append(s_v2)

                def s_h1():
                    ps_h = psp.tile([128, 512], F32, name=f"ph1{g}", tag="ps")
                    nc.tensor.matmul(ps_h, CE[4], ctx["E2"], start=True,
                                     stop=False)
                    nc.tensor.matmul(ps_h, CE[3], ctx["E1"], start=False,
                                     stop=True)
                    H1 = work.tile([128, 512], BF16, name=f"H1{g}", tag="H1e")
                    cb_stc(H1, ps_h, 1.0, IP_e2, ADD)
                    ctx["H1e"] = H1
                st.append(s_h1)

                def s_e():
                    ps_e = psp.tile([128, 512], F32, name=f"pe{g}", tag="ps")
                    nc.tensor.matmul(ps_e, CE[1], ctx["E1"], start=True,
                                     stop=False)
                    quad16(ps_e, ctx["E2"], ctx["H1e"], first=False,
                           last=True)
                    Et = work.tile([128, 512], F32, name=f"E{g}", tag="E")
                    cb_stc(Et, ps_e, 1.0, IP_e0, ADD)
                    ctx["E"] = Et
                st.append(s_e)

                def s_sq():
                    ps_o = psp.tile([128, 512], F32, name=f"po{g}", tag="ps")
                    quad16(ps_o, ctx["E"], ctx["E"])
                    Ot = work.tile([128, 512], F32, name=f"O{g}", tag="O")
                    cb_act(Ot, ps_o)
                    ctx["He"] = Ot
                st.append(s_sq)

                def s_out():
                    He3 = ctx["He"].rearrange("r (p c) -> r p c", p=8)
                    nc.sync.dma_start(o_r[g, 0], He3[0:64])
                    nc.sync.dma_start(o_r[g, 1], He3[64:128])
                st.append(s_out)
                return st

            pipe = int(os.environ.get("K_PIPE", 6))
            for g0 in range(0, ngrp, pipe):
                lanes = [group_stages(g) for g in range(g0, min(g0 + pipe,
                                                                ngrp))]
                maxlen = max(len(x) for x in lanes)
                for si in range(maxlen):
                    for lane in lanes:
                        if si < len(lane):
                            lane[si]()

    nc.compile()
    return nc, blob


_CACHE = {}


def kernel(X, running_mean):
    key = running_mean.tobytes()[:256]
    if key not in _CACHE:
        hp = _host_prep(np.asarray(running_mean, dtype=np.float32))
        _CACHE[key] = _build_kernel(hp)
    nc, blob = _CACHE[key]

    X = np.ascontiguousarray(np.asarray(X, dtype=np.float32))
    in_maps = [{"x": X[i * SPC:(i + 1) * SPC], "cst": blob}
               for i in range(NCORES)]
    res = bass_utils.run_bass_kernel_spmd(
        nc, in_maps, core_ids=list(range(NCORES)),
        trace=bool(int(os.environ.get("K_TRACE", "0"))))
    out = np.concatenate([res.results[i]["out"] for i in range(NCORES)],
                         axis=0)
    kernel.last_exec_time_ns = res.exec_time_ns
    return out.astype(np.float32)


kernel.last_exec_time_ns = None


# revision 18
# speedup vs baseline: 1.0791x; 1.0791x over previous
"""BarycenterNorm (eval mode) Trainium2 kernel.

Math: the reference chain collapses to out_i = exp(T log(T X_i T^T) T^T)
with T = chol(B^-1).T (verified vs reference to 5e-9). With X and L
symmetric, both congruences avoid per-sample transposes on the PE
(out = lhsT.T @ rhs):
  A1 = X_i T^T    per-sample stationary X_i, shared moving T^T  (f32 quad)
  Y  = T A1       shared stationary diag(aT^T,aT^T), f32r       -> W
  W  = a*Y + b*I  spectrum in [-1,1]
  2L = cheb2-Clenshaw(W)    all-bf16, two-level Chebyshev basis:
       p(W) = sum_j Q_j(W) T_j(z), z = T_s(W); doubled recurrence
       Bt_j = 2(z Bt_{j+1} + Q_j) - Bt_{j+2} keeps the x2 out of Q.
  A2 = (2L) T^T   per-sample stationary, f32 quad
  V  = T A2/(2zb) shared f32r; spectrum [-1,1]
  out = p_exp(V)  monomial PS (coeffs O(1)), bf16

Layout: 16-sample groups; tiles [128,512]: samples 16g..16g+7 in
partitions 0-63, 16g+8..16g+15 in 64-127, each a [64,64] block along
free. Per-sample matmuls are 64x64 quadrant matmuls at (0,0)/(64,64).
Coefficient injection: K=128 (c*I) matmuls accumulate in PSUM; they must
be emitted BEFORE the quadrant matmuls of the same bank (HW quirk:
full-K accumulation after quadrant-mode start=True writes corrupts).
"""
import os
import sys

import numpy as np

sys.path.insert(0, "/opt/trn_rl_repo")

import concourse.bacc as bacc  # noqa: E402
import concourse.tile as tile  # noqa: E402
from concourse import mybir  # noqa: E402
from concourse import bass_utils  # noqa: E402

try:
    import axon_profile_shim  # noqa: F401
except Exception:
    pass

F32 = mybir.dt.float32
F32R = mybir.dt.float32r
BF16 = mybir.dt.bfloat16

C = 64
BATCH = 8192
NCORES = 8
SPC = BATCH // NCORES
NGRP = SPC // 16

N_LOG = int(os.environ.get("K_NLOG", 13))
S_LOG = int(os.environ.get("K_SLOG", 4))
N_EXP = int(os.environ.get("K_NEXP", 4))
S_EXP = int(os.environ.get("K_SEXP", 3))
A_MARGIN = 0.98
B_MARGIN = 6.5

MULT = mybir.AluOpType.mult
ADD = mybir.AluOpType.add
SUB = mybir.AluOpType.subtract
COPYF = mybir.ActivationFunctionType.Copy


def _cheb_coeffs(f, lo, hi, deg):
    k = np.arange(deg + 1)
    nw = np.cos((2 * k + 1) * np.pi / (2 * (deg + 1)))
    nx = 0.5 * (hi - lo) * nw + 0.5 * (lo + hi)
    return np.polynomial.chebyshev.chebfit(nw, f(nx), deg)


def _solve_two_level(a, s):
    """p = sum_{j,i} b[j,i] T_i(w) T_{js}(w); triangular solve."""
    n = len(a) - 1
    r = n // s
    rem = a.astype(np.float64).copy()
    b = np.zeros((r + 1, s))
    for j in range(r, -1, -1):
        for i in range(min(s - 1, n - j * s), 0, -1):
            m = j * s + i
            if j == 0:
                b[j, i] = rem[m]
                rem[m] = 0.0
            else:
                coef = 2.0 * rem[m]
                b[j, i] = coef
                rem[m] = 0.0
                rem[abs(j * s - i)] -= coef / 2.0
        b[j, 0] = rem[j * s]
        rem[j * s] = 0.0
    assert np.abs(rem).max() < 1e-10
    return b


def _host_prep(running_mean):
    B = running_mean[0].astype(np.float64)
    sev = np.linalg.eigvalsh(B)
    T = np.linalg.cholesky(np.linalg.inv(B)).T  # upper; T^T T = B^-1
    a = A_MARGIN / sev[-1]
    b = B_MARGIN / sev[0]
    alpha = 2.0 / (b - a)
    beta = -(a + b) / (b - a)
    zb = max(abs(np.log(a)), abs(np.log(b))) / sev[0]
    b_log = _solve_two_level(_cheb_coeffs(np.log, a, b, N_LOG), S_LOG)
    c_exp = np.polynomial.chebyshev.cheb2poly(
        _cheb_coeffs(lambda v: np.exp(zb * v / 2.0), -1.0, 1.0, N_EXP))
    return dict(T=T, alpha=alpha, beta=beta, zb=zb, b_log=b_log,
                c_exp=c_exp, r_log=N_LOG // S_LOG, r_exp=N_EXP // S_EXP)


def _build_consts(hp):
    T = hp["T"].astype(np.float32)
    Tt = np.ascontiguousarray(T.T)
    alpha = np.float32(hp["alpha"])
    zb = np.float32(hp["zb"])
    I64 = np.eye(64, dtype=np.float32)

    slabs = {}
    cols = []

    def add(name, arr):
        c0 = sum(a.shape[1] for a in cols)
        cols.append(np.ascontiguousarray(arr, dtype=np.float32))
        slabs[name] = (c0, c0 + arr.shape[1])

    dTT = np.zeros((128, 128), np.float32)
    dTT[:64, :64] = Tt
    dTT[64:, 64:] = Tt
    add("S1", alpha * dTT)
    add("S2", dTT / (2.0 * zb))
    add("Tmov", np.concatenate([Tt, Tt], axis=0))  # [128, 64]

    ipat = np.zeros((128, 512), np.float32)
    for j in range(8):
        ipat[:64, 64 * j:64 * j + 64] = I64
        ipat[64:, 64 * j:64 * j + 64] = I64

    add("IP_beta", np.float32(hp["beta"]) * ipat)
    add("IP_unit", ipat)

    ce = hp["c_exp"]
    add("IP_e2", np.float32(ce[2]) * ipat)
    add("IP_e0", np.float32(ce[0]) * ipat)

    I128 = np.eye(128, dtype=np.float32)
    b_log = hp["b_log"]
    add("IP_br0", np.float32(2.0 * b_log[hp["r_log"], 0]) * ipat)
    for j in range(hp["r_log"] + 1):
        for i in range(S_LOG):
            v = b_log[j, i] * (2.0 if j == 0 else 1.0)
            if i == 0:
                continue  # j==r rides the copy-back; others go via DVE
            if abs(v) > 3e-4:
                add(f"CL{j}_{i}", np.float32(v) * I128)
    for i in (4, 3, 1):
        add(f"CE_{i}", np.float32(ce[i]) * I128)

    return np.concatenate(cols, axis=1), slabs, None


def _build_kernel(hp, ngrp=None):
    if ngrp is None:
        ngrp = int(os.environ.get("K_NGRP", NGRP))
    blob, slabs, _ = _build_consts(hp)
    r_log, r_exp = hp["r_log"], hp["r_exp"]

    b_log_v = hp["b_log"]
    nc = bacc.Bacc("TRN2", target_bir_lowering=False, debug=False)
    x_d = nc.dram_tensor("x", [SPC, C, C], F32, kind="ExternalInput").ap()
    cst_d = nc.dram_tensor("cst", [128, blob.shape[1]], F32,
                           kind="ExternalInput").ap()
    out_d = nc.dram_tensor("out", [SPC, C, C], F32, kind="ExternalOutput").ap()

    x_r = x_d.rearrange("(g two p) r c -> g two r p c", g=NGRP, two=2)
    o_r = out_d.rearrange("(g two p) r c -> g two r p c", g=NGRP, two=2)

    with tile.TileContext(nc) as tc:
        with tc.tile_pool(name="csts", bufs=1) as csts, \
             tc.tile_pool(name="work", bufs=int(os.environ.get("K_WBUFS", 7)) ) as work, \
             tc.tile_pool(name="psp", bufs=int(os.environ.get("K_PSBUFS", 8)), space="PSUM") as psp:

            _cst_cache = {}

            def cslab(name, dtype=F32):
                c0, c1 = slabs[name]
                if name in _cst_cache:
                    t = _cst_cache[name]
                else:
                    t = csts.tile([128, c1 - c0], F32, name=f"cst_{name}",
                                  tag=f"cst_{name}")
                    nc.sync.dma_start(t, cst_d[:, c0:c1])
                    _cst_cache[name] = t
                if dtype != F32:
                    tb = csts.tile([128, c1 - c0], dtype,
                                   name=f"cstb_{name}_{dtype}",
                                   tag=f"cstb_{name}_{dtype}")
                    nc.vector.tensor_copy(tb, t)
                    return tb
                return t

            S1 = cslab("S1", F32R)
            S2 = cslab("S2", F32R)
            Tmov = cslab("Tmov")
            IPb = cslab("IP_beta")
            IPu = cslab("IP_unit")
            IPub = cslab("IP_unit", BF16)
            IP_e2 = cslab("IP_e2")
            IP_e0 = cslab("IP_e0")
            IP_br0 = cslab("IP_br0")
            CL = {}
            for j in range(r_log + 1):
                for i in range(S_LOG):
                    if f"CL{j}_{i}" in slabs:
                        CL[(j, i)] = cslab(f"CL{j}_{i}", BF16)
            CE = {i: cslab(f"CE_{i}", BF16) for i in (4, 3, 1)}


            def cb_stc(out, ps, scalar, in1, op1):
                nc.vector.scalar_tensor_tensor(out, ps, scalar, in1,
                                               MULT, op1)

            def cb_act(out, ps, scale=1.0):
                nc.scalar.activation(out, ps, COPYF, scale=scale)

            def quad16(ps, stat_tile, mov_tile, first=True, last=True,
                       mov64=None):
                # interleave row-halves so each LDWEIGHTS overlaps the
                # other half's in-flight matmul (different row_grp)
                n = 0
                for j in range(8):
                    for h in (0, 64):
                        sl = slice(64 * j, 64 * j + 64)
                        mov = (mov64[h:h + 64, 0:64] if mov64 is not None
                               else mov_tile[h:h + 64, sl])
                        nc.tensor.matmul(
                            ps[h:h + 64, sl], stat_tile[h:h + 64, sl], mov,
                            start=first, stop=(last and n == 15),
                            tile_position=(h, h))
                        n += 1

            def group_stages(g):
                st = []
                ctx = {}

                def s_load():
                    Xt = work.tile([128, 512], F32, name=f"X{g}", tag="X")
                    Xt3 = Xt.rearrange("r (p c) -> r p c", p=8)
                    nc.sync.dma_start(Xt3[0:64], x_r[g, 0])
                    nc.sync.dma_start(Xt3[64:128], x_r[g, 1])
                    ps_a = psp.tile([128, 512], F32, name=f"psa{g}", tag="ps")
                    quad16(ps_a, Xt, None, mov64=Tmov)
                    A1 = work.tile([128, 512], F32R, name=f"A1_{g}", tag="A1")
                    cb_act(A1, ps_a)
                    ctx["A1"] = A1
                st.append(s_load)

                def s_w():
                    ps_y = psp.tile([128, 512], F32, name=f"psy{g}", tag="ps")
                    nc.tensor.matmul(ps_y, S1, ctx["A1"], start=True,
                                     stop=True)
                    Wt = work.tile([128, 512], BF16, name=f"W{g}", tag="W")
                    cb_stc(Wt, ps_y, 1.0, IPb, ADD)
                    ctx["T1"] = Wt
                st.append(s_w)

                for k in range(2, S_LOG + 1):
                    def s_t(k=k):
                        Wt = ctx["T1"]
                        ps_t = psp.tile([128, 512], F32, name=f"pst{g}_{k}",
                                        tag="ps")
                        quad16(ps_t, Wt, ctx[f"T{k - 1}"] if k > 2 else Wt)
                        Tk = work.tile([128, 512], BF16, name=f"T{g}_{k}",
                                       tag=f"T{k}")
                        sub_src = IPu if k == 2 else ctx[f"T{k - 2}"]
                        cb_stc(Tk, ps_t, 2.0, sub_src, SUB)
                        ctx[f"T{k}"] = Tk
                    st.append(s_t)

                def emit_Q(ps_h, j, quad_mov=None):
                    # i>=1 terms on PE (cI matmuls, emitted before quads);
                    # the b_{j,0}*I ride goes to DVE as an in-place PSUM add
                    # after the PE matmuls (PE is the bottleneck engine).
                    first = True
                    for i in range(1, S_LOG):
                        if (j, i) in CL:
                            nc.tensor.matmul(ps_h, CL[(j, i)], ctx[f"T{i}"],
                                             start=first, stop=False)
                            first = False
                    assert not first
                    if quad_mov is not None:
                        quad16(ps_h, ctx[f"T{S_LOG}"], quad_mov, first=False,
                               last=True)
                    v0 = b_log_v[j, 0] * (2.0 if j == 0 else 1.0)
                    if j != r_log and abs(v0) > 3e-4:
                        nc.vector.scalar_tensor_tensor(
                            ps_h, IPu, float(v0), ps_h, MULT, ADD)

                def s_br():
                    ps_h = psp.tile([128, 512], F32, name=f"psq{g}_{r_log}",
                                    tag="ps")
                    emit_Q(ps_h, r_log)
                    Br = work.tile([128, 512], BF16, name=f"B{g}_{r_log}",
                                   tag=f"B{r_log}")
                    cb_stc(Br, ps_h, 2.0, IP_br0, ADD)
                    ctx[f"B{r_log}"] = Br
                st.append(s_br)

                for j in range(r_log - 1, 0, -1):
                    def s_bj(j=j):
                        ps_h = psp.tile([128, 512], F32, name=f"psq{g}_{j}",
                                        tag="ps")
                        emit_Q(ps_h, j, quad_mov=ctx[f"B{j + 1}"])
                        Bj = work.tile([128, 512], BF16, name=f"B{g}_{j}",
                                       tag=f"B{j}")
                        if f"B{j + 2}" in ctx:
                            cb_stc(Bj, ps_h, 2.0, ctx[f"B{j + 2}"], SUB)
                        else:
                            cb_act(Bj, ps_h, scale=2.0)
                        ctx[f"B{j}"] = Bj
                    st.append(s_bj)

                def s_l():
                    ps_h = psp.tile([128, 512], F32, name=f"psq{g}_0",
                                    tag="ps")
                    emit_Q(ps_h, 0, quad_mov=ctx["B1"])
                    Lt = work.tile([128, 512], F32, name=f"L{g}", tag="L")
                    cb_stc(Lt, ps_h, 1.0, ctx["B2"], SUB)
                    ctx["L"] = Lt
                st.append(s_l)

                def s_a2():
                    ps_b = psp.tile([128, 512], F32, name=f"psb{g}", tag="ps")
                    quad16(ps_b, ctx["L"], None, mov64=Tmov)
                    A2 = work.tile([128, 512], F32R, name=f"A2_{g}", tag="A2")
                    cb_act(A2, ps_b)
                    ctx["A2"] = A2
                st.append(s_a2)

                def s_v():
                    ps_v = psp.tile([128, 512], F32, name=f"psv{g}", tag="ps")
                    nc.tensor.matmul(ps_v, S2, ctx["A2"], start=True,
                                     stop=True)
                    Vt = work.tile([128, 512], BF16, name=f"V{g}", tag="V")
                    cb_act(Vt, ps_v)
                    ctx["E1"] = Vt
                st.append(s_v)

                def s_v2():
                    ps_2 = psp.tile([128, 512], F32, name=f"pv2{g}", tag="ps")
                    quad16(ps_2, ctx["E1"], ctx["E1"])
                    V2 = work.tile([128, 512], BF16, name=f"V2{g}", tag="V2")
                    cb_act(V2, ps_2)
                    ctx["E2"] = V2
                st.append(s_v2)

                def s_h1():
                    ps_h = psp.tile([128, 512], F32, name=f"ph1{g}", tag="ps")
                    nc.tensor.matmul(ps_h, CE[4], ctx["E2"], start=True,
                                     stop=False)
                    nc.tensor.matmul(ps_h, CE[3], ctx["E1"], start=False,
                                     stop=True)
                    H1 = work.tile([128, 512], BF16, name=f"H1{g}", tag="H1e")
                    cb_stc(H1, ps_h, 1.0, IP_e2, ADD)
                    ctx["H1e"] = H1
                st.append(s_h1)

                def s_e():
                    ps_e = psp.tile([128, 512], F32, name=f"pe{g}", tag="ps")
                    nc.tensor.matmul(ps_e, CE[1], ctx["E1"], start=True,
                                     stop=False)
                    quad16(ps_e, ctx["E2"], ctx["H1e"], first=False,
                           last=True)
                    Et = work.tile([128, 512], F32, name=f"E{g}", tag="E")
                    cb_stc(Et, ps_e, 1.0, IP_e0, ADD)
                    ctx["E"] = Et
                st.append(s_e)

                def s_sq():
                    ps_o = psp.tile([128, 512], F32, name=f"po{g}", tag="ps")
                    quad16(ps_o, ctx["E"], ctx["E"])
                    Ot = work.tile([128, 512], F32, name=f"O{g}", tag="O")
                    cb_act(Ot, ps_o)
                    ctx["He"] = Ot
                st.append(s_sq)

                def s_out():
                    He3 = ctx["He"].rearrange("r (p c) -> r p c", p=8)
                    nc.sync.dma_start(o_r[g, 0], He3[0:64])
                    nc.sync.dma_start(o_r[g, 1], He3[64:128])
                st.append(s_out)
                return st

            pipe = int(os.environ.get("K_PIPE", 6))
            for g0 in range(0, ngrp, pipe):
                lanes = [group_stages(g) for g in range(g0, min(g0 + pipe,
                                                                ngrp))]
                maxlen = max(len(x) for x in lanes)
                for si in range(maxlen):
                    for lane in lanes:
                        if si < len(lane):
                            lane[si]()

    nc.compile()
    return nc, blob


_CACHE = {}


def kernel(X, running_mean):
    key = running_mean.tobytes()[:256]
    if key not in _CACHE:
        hp = _host_prep(np.asarray(running_mean, dtype=np.float32))
        _CACHE[key] = _build_kernel(hp)
    nc, blob = _CACHE[key]

    X = np.ascontiguousarray(np.asarray(X, dtype=np.float32))
    in_maps = [{"x": X[i * SPC:(i + 1) * SPC], "cst": blob}
               for i in range(NCORES)]
    res = bass_utils.run_bass_kernel_spmd(
        nc, in_maps, core_ids=list(range(NCORES)),
        trace=bool(int(os.environ.get("K_TRACE", "0"))))
    out = np.concatenate([res.results[i]["out"] for i in range(NCORES)],
                         axis=0)
    kernel.last_exec_time_ns = res.exec_time_ns
    return out.astype(np.float32)


kernel.last_exec_time_ns = None
